# revision 1
# baseline (speedup 1.0000x reference)
"""Trainium2 Bass kernel for nn_Attention_49074296324413.

Data-parallel over batch: core b handles batch element b of
  kv = dw3x3(conv1x1(x, w_kv)); k, v = split(kv); k = avgpool2x2(k)
  q  = conv3x3(conv1x1(y, w_q))
  out = conv1x1(softmax(norm(q) @ norm(k).T * temp) @ v, w_proj)

Precision split (driven by where per-element quantization noise survives to
the output): the q/k path runs fp8(e4m3) with DoubleRow perf mode -- its
noise is filtered by L2 normalization, the 1024-px inner products and
softmax; the v path (A-v, v-depthwise, fused attn@v+proj) stays bf16 since
its per-element error passes straight through to the output.

Tensor-engine structure:
 - C (q1 = Wq@y) and A-k (k1 = Wk@x) in fp8 DR: contraction tile pairs per
   pass via [128,2,N] APs (pair stride 16B-aligned, tile-block layouts).
 - D (full 3x3 conv on q1): fp8 DR over (ci, tap) block pairs sharing dx,
   using overlapping strided 4D moving APs into a 48-elem-pitch padded
   image -- 15 passes instead of 27.
 - k depthwise+pool folded to a 4x4-stride-2 conv, run as fp8-DR diagonal
   matmuls with vertical tap pairs (80B-pitch padded rows): 8 passes for
   16 taps.
 - v depthwise: bf16 diagonal matmuls for 7 taps; taps (1,0) and (1,1) are
   computed on the scalar+vector engines and folded into the PSUM
   evacuation (scalar_tensor_tensor).
 - attn@v and the projection are fused into one dense matmul via per-head
   M_h = attn_h.T @ w_proj[:, head].T stacking; QK/softmax/M in bf16.

Scheduling: input DMAs are queue-ordered to match consumption (xb chunks
first), phases emit C -> A-v -> A-k -> k-pool -> norms/transposes -> D ->
q-norms -> q3 transposes -> QK+softmax+M -> (B1 <-> H interleaved by pixel
group, with output DMA streaming behind each group).  All weight scales are
powers of two chosen host-side; normalization washes them out on the q/k
side and plain-copy evacuation keeps the v side in natural scale.
"""
import numpy as np
import ml_dtypes

import concourse.bass as bass
import concourse.tile as tile
from concourse import bacc, mybir
from concourse.ap import AP
from concourse.bass_utils import run_bass_kernel_spmd

dt = mybir.dt
BF = dt.bfloat16
F8 = dt.float8e4
F32 = dt.float32
AF = mybir.ActivationFunctionType
OP = mybir.AluOpType
DR = mybir.MatmulPerfMode.DoubleRow

DIM = 384
HEADS = 8
HC = DIM // HEADS          # 48 channels per head
CT = DIM // 128            # 3 channel tiles
H = 64                     # x spatial
NPIX = H * H               # 4096
H2 = 32                    # y spatial
NPIX2 = H2 * H2            # 1024
PW = 66                    # padded rows for 64-grid
RW = 80                    # padded row pitch (bytes/elems) for 64-grid
PW2 = 34                   # padded rows for 32-grid
RW2 = 48                   # padded row pitch for 32-grid

BF_NP = ml_dtypes.bfloat16
F8_NP = ml_dtypes.float8_e4m3

# power-of-2 operand scales for the fp8 q/k path (see module docstring)
S_A = 2.0 ** 6     # w_k, w_q
S_DW = 2.0 ** 11   # w4k, wqdw


def _ap(base: AP, off: int, dims):
    """Custom strided AP into a tile's free space: dims = [[stride, n], ...]."""
    return AP(tensor=base.tensor, offset=base.offset + off,
              ap=[[base.ap[0][0], base.ap[0][1]]] + [list(d) for d in dims])


def build_program(dbg: bool = False):
    nc = bacc.Bacc("TRN2", target_bir_lowering=False, debug=False)

    y3_d = nc.dram_tensor("y3", (128, CT * NPIX2), F8, kind="ExternalInput")
    wkT_d = nc.dram_tensor("wkT3", (128, CT * DIM), F8, kind="ExternalInput")
    wvT_d = nc.dram_tensor("wvT3", (128, CT * DIM), BF, kind="ExternalInput")
    xb_d = nc.dram_tensor("xb3", (128, CT * NPIX), BF, kind="ExternalInput")
    wqT_d = nc.dram_tensor("wqT3", (128, CT * DIM), F8, kind="ExternalInput")
    wqdwT_d = nc.dram_tensor("wqdwT3", (128, CT * 9 * DIM), F8, kind="ExternalInput")
    w3v_d = nc.dram_tensor("w3vc", (128, CT * 9), F32, kind="ExternalInput")
    x8_d = nc.dram_tensor("x8", (128, CT * NPIX), F8, kind="ExternalInput")
    dgv_d = nc.dram_tensor("dgv", (128, CT * 9 * 128), BF, kind="ExternalInput")
    dgk_d = nc.dram_tensor("dgk", (128, CT * 2048), F8, kind="ExternalInput")
    wpT_d = nc.dram_tensor("wpT", (DIM, DIM), BF, kind="ExternalInput")
    temp_d = nc.dram_tensor("temp_col", (DIM, 1), F32, kind="ExternalInput")
    idn_d = nc.dram_tensor("idn", (128, 128), BF, kind="ExternalInput")

    out_d = nc.dram_tensor("out", (DIM, NPIX), BF, kind="ExternalOutput")
    dbg_d = {}
    if dbg:
        dbg_d["vdw"] = nc.dram_tensor("dbg_vdw", (DIM, NPIX), BF, kind="ExternalOutput")
        dbg_d["kpn"] = nc.dram_tensor("dbg_kpn", (DIM, NPIX2), BF, kind="ExternalOutput")
        dbg_d["q3n"] = nc.dram_tensor("dbg_q3n", (DIM, NPIX2), BF, kind="ExternalOutput")
        dbg_d["att"] = nc.dram_tensor("dbg_att", (HEADS * HC, HC), BF, kind="ExternalOutput")
        dbg_d["mst"] = nc.dram_tensor("dbg_mst", (128, CT * DIM), BF, kind="ExternalOutput")

    with tile.TileContext(nc) as tc:
        _emit(nc, tc, xb_d, x8_d, y3_d, wkT_d, wvT_d, wqT_d, wqdwT_d, w3v_d,
              dgv_d, dgk_d, wpT_d, temp_d, idn_d, out_d, dbg_d)
    nc.compile()
    return nc


def _emit(nc, tc, xb_d, x8_d, y3_d, wkT_d, wvT_d, wqT_d, wqdwT_d, w3v_d,
          dgv_d, dgk_d, wpT_d, temp_d, idn_d, out_d, dbg_d):
    from contextlib import ExitStack
    ctx = ExitStack()

    cst = ctx.enter_context(tc.tile_pool(name="cst", bufs=1))
    big = ctx.enter_context(tc.tile_pool(name="big", bufs=1))
    wrk = ctx.enter_context(tc.tile_pool(name="wrk", bufs=2))
    osb = ctx.enter_context(tc.tile_pool(name="osb", bufs=4))
    ps_a = ctx.enter_context(tc.tile_pool(name="ps_a", bufs=4, space="PSUM"))
    ps_d = ctx.enter_context(tc.tile_pool(name="ps_d", bufs=2, space="PSUM"))
    ps_m = ctx.enter_context(tc.tile_pool(name="ps_m", bufs=2, space="PSUM"))

    # ---------------- input DMAs ----------------
    # First-needed first per queue; aggregate DMA is the startup bottleneck:
    # A-v streams xb chunks as they land, then A-k (x8), kpool (dgk), D, B1.
    idn_t = cst.tile([128, 128], BF, tag="idn", name="idn")
    nc.sync.dma_start(idn_t[:], idn_d.ap())
    xb_t = cst.tile([128, CT, NPIX], BF, tag="xb3", name="xb3")
    xbv = xb_d.ap().rearrange("p (a b) -> p a b", a=CT)
    nc.sync.dma_start(xb_t[:, :, 0:1024], xbv[:, :, 0:1024])
    nc.gpsimd.dma_start(xb_t[:, :, 1024:2048], xbv[:, :, 1024:2048])
    y3_t = cst.tile([128, CT * NPIX2], F8, tag="y3", name="y3")
    nc.scalar.dma_start(y3_t[:], y3_d.ap())
    wqT_t = cst.tile([128, CT * DIM], F8, tag="wqT", name="wqT")
    nc.scalar.dma_start(wqT_t[:], wqT_d.ap())
    wvT_t = cst.tile([128, CT * DIM], BF, tag="wvT", name="wvT")
    nc.scalar.dma_start(wvT_t[:], wvT_d.ap())
    wkT_t = cst.tile([128, CT * DIM], F8, tag="wkT", name="wkT")
    nc.scalar.dma_start(wkT_t[:], wkT_d.ap())
    nc.sync.dma_start(xb_t[:, :, 2048:3072], xbv[:, :, 2048:3072])
    nc.gpsimd.dma_start(xb_t[:, :, 3072:4096], xbv[:, :, 3072:4096])
    x8_t = cst.tile([128, CT, NPIX], F8, tag="x8", name="x8")
    x8v = x8_d.ap().rearrange("p (a b) -> p a b", a=CT)
    nc.sync.dma_start(x8_t[:, :, 0:2048], x8v[:, :, 0:2048])
    nc.gpsimd.dma_start(x8_t[:, :, 2048:4096], x8v[:, :, 2048:4096])
    w3v_t = cst.tile([128, CT * 9], F32, tag="w3vc", name="w3vc")
    nc.scalar.dma_start(w3v_t[:], w3v_d.ap())
    tempc_t = []
    for ct in range(CT):
        t = cst.tile([128, 1], F32, tag=f"tempc{ct}", name=f"tempc{ct}")
        nc.scalar.dma_start(t[:], temp_d.ap()[128 * ct:128 * (ct + 1), :])
        tempc_t.append(t)
    dgk_t = cst.tile([128, CT * 2048], F8, tag="dgk", name="dgk")
    nc.scalar.dma_start(dgk_t[:], dgk_d.ap())
    wqdwT_t = cst.tile([128, CT * 9 * DIM], F8, tag="wqdwT", name="wqdwT")
    nc.sync.dma_start(wqdwT_t[:], wqdwT_d.ap())
    dgv_t = cst.tile([128, CT, 9, 128], BF, tag="dgv", name="dgv")
    nc.gpsimd.dma_start(dgv_t[:].rearrange("p a b c -> p (a b c)"), dgv_d.ap())
    wpT_t = []
    for h in range(HEADS):
        t = cst.tile([HC, DIM], BF, tag=f"wpT{h}", name=f"wpT{h}")
        nc.gpsimd.dma_start(t[:], wpT_d.ap()[HC * h:HC * (h + 1), :])
        wpT_t.append(t)
    eps_col = cst.tile([128, 1], F32, tag="eps_col", name="eps_col")
    nc.vector.memset(eps_col[:], 1e-24)
    zero_col = cst.tile([128, 1], F32, tag="zero_col", name="zero_col")
    nc.vector.memset(zero_col[:], 0.0)

    # ---------------- padded image buffers (zero borders) ----------------
    kpad = big.tile([128, CT, PW, RW], F8, tag="kpad", name="kpad")
    vpad = big.tile([128, CT, PW, RW], BF, tag="vpad", name="vpad")
    q1pad = big.tile([128, CT, PW2, RW2], F8, tag="q1pad", name="q1pad")
    for ct in range(CT):
        for t, pw in ((kpad, PW), (vpad, PW), (q1pad, PW2)):
            eng = nc.vector if ct % 2 == 0 else nc.gpsimd
            eng.memset(t[:, ct, 0, :], 0.0)
            eng.memset(t[:, ct, pw - 1, :], 0.0)
            eng.memset(t[:, ct, 1:pw - 1, 0:1], 0.0)
            eng.memset(t[:, ct, 1:pw - 1, pw - 1:pw], 0.0)
    kpadf = kpad[:].rearrange("p a b c -> p (a b c)")
    vpadf = vpad[:].rearrange("p a b c -> p (a b c)")
    q1padf = q1pad[:].rearrange("p a b c -> p (a b c)")
    CB = PW * RW       # 5280 elems per channel-tile block
    CB2 = PW2 * RW2    # 1632

    # ---------------- phase C: q1 (emitted inside A-v schedule) = W_q @ y -> q1pad (fp8, x2^6) ----------
    def _emit_c():
        for co in range(CT):
            for j in range(2):
                ps = ps_a.tile([128, 512], F32, tag="ps_a", name="ps_a")
                nc.tensor.matmul(
                    ps[:],
                    _ap(wqT_t[:], co * 128, [[DIM, 2], [1, 128]]),
                    _ap(y3_t[:], j * 512, [[NPIX2, 2], [1, 512]]),
                    start=True, stop=False, perf_mode=DR)
                nc.tensor.matmul(
                    ps[:],
                    _ap(wqT_t[:], 2 * DIM + co * 128, [[1, 128]]),
                    _ap(y3_t[:], 2 * NPIX2 + j * 512, [[1, 512]]),
                    start=False, stop=True)
                nc.scalar.copy(q1pad[:, co, 1 + 16 * j:17 + 16 * j, 1:33],
                               ps[:].rearrange("p (a b) -> p a b", a=16))

    # ---------------- phase A: kv1 = W_kv @ x (bf16) ----------------------
    def a_block(co, g):
        dst = kpad if co < CT else vpad
        ct = co % CT
        for jj in range(2):
            c = 2 * g + jj
            ps = ps_a.tile([128, 512], F32, tag="ps_a", name="ps_a")
            if co < CT:
                nc.tensor.matmul(
                    ps[:],
                    _ap(wkT_t[:], ct * 128, [[DIM, 2], [1, 128]]),
                    _ap(x8_t[:].rearrange("p a b -> p (a b)"), c * 512,
                        [[NPIX, 2], [1, 512]]),
                    start=True, stop=False, perf_mode=DR)
                nc.tensor.matmul(
                    ps[:],
                    _ap(wkT_t[:], 2 * DIM + ct * 128, [[1, 128]]),
                    x8_t[:, 2, 512 * c:512 * (c + 1)],
                    start=False, stop=True)
            else:
                for ci in range(CT):
                    nc.tensor.matmul(
                        ps[:],
                        _ap(wvT_t[:], ci * DIM + ct * 128, [[1, 128]]),
                        xb_t[:, ci, 512 * c:512 * (c + 1)],
                        start=(ci == 0), stop=(ci == CT - 1))
            eng = nc.scalar.copy if (co + c) % 2 == 0 else nc.vector.tensor_copy
            eng(dst[:, ct, 1 + 8 * c:9 + 8 * c, 1:65],
                ps[:].rearrange("p (a b) -> p a b", a=8))

    # ---------------- phase B2: k depthwise+pool on PE (fp8 diag DR) ------
    kp_t = [big.tile([128, NPIX2], BF, tag=f"kp{ct}", name=f"kp{ct}") for ct in range(CT)]

    def kpool_block(ct):
        for i0 in (0, 16):          # output row halves (512 px each)
            ps = ps_d.tile([128, 512], F32, tag="ps_d", name="ps_d")
            for ux in range(4):
                for pp in range(2):  # uy pairs (0,1), (2,3)
                    nc.tensor.matmul(
                        ps[:],
                        _ap(dgk_t[:], ct * 2048 + ux * 512 + pp * 256,
                            [[128, 2], [1, 128]]),
                        _ap(kpadf, ct * CB + (2 * i0 + 2 * pp) * RW + ux,
                            [[RW, 2], [2 * RW, 16], [2, 32]]),
                        start=(ux == 0 and pp == 0),
                        stop=(ux == 3 and pp == 1), perf_mode=DR)
            nc.vector.tensor_copy(kp_t[ct][:, 512 * (i0 // 16):512 * (i0 // 16 + 1)],
                                  ps[:])

    for g in range(4):
        for co in (3, 4, 5):
            a_block(co, g)
        if g == 0:
            _emit_c()
    for g in range(4):
        for co in (0, 1, 2):
            a_block(co, g)
    kpool_block(0)
    kpool_block(1)
    kpool_block(2)

    # ---------------- phase B3: k norms (scale washes out) ----------------
    for ct in range(CT):
        sq = wrk.tile([128, NPIX2], BF, tag="sqk", name="sqk")
        nrm2 = wrk.tile([128, 1], F32, tag="nrm2k", name="nrm2k")
        nc.scalar.activation(sq[:], kp_t[ct][:], AF.Square, bias=zero_col[:],
                             accum_out=nrm2[:])
        nrm = wrk.tile([128, 1], F32, tag="nrmk", name="nrmk")
        nc.scalar.activation(nrm[:], nrm2[:], AF.Sqrt, bias=eps_col[:])
        inv = wrk.tile([128, 1], F32, tag="invk", name="invk")
        nc.vector.reciprocal(inv[:], nrm[:])
        nc.vector.tensor_scalar_mul(kp_t[ct][:], kp_t[ct][:], inv[:])
        if "kpn" in dbg_d:
            nc.sync.dma_start(dbg_d["kpn"].ap()[128 * ct:128 * (ct + 1), :], kp_t[ct][:])

    # ---------------- phase B4: kpT via PE transpose ----------------
    kpT = [big.tile([128, DIM], BF, tag=f"kpT{pt}", name=f"kpT{pt}") for pt in range(8)]
    for ct in range(CT):
        for pt in range(8):
            pst = ps_m.tile([128, 128], BF, tag="ps_m", name="ps_m")
            nc.tensor.transpose(pst[:], kp_t[ct][:, 128 * pt:128 * (pt + 1)], idn_t[:])
            eng = (nc.vector.tensor_copy, nc.scalar.copy)[(ct + pt) % 2]
            eng(kpT[pt][:, 128 * ct:128 * (ct + 1)], pst[:])

    # ---------------- phase D: q3 full 3x3 conv (fp8 DR pairs) ------------
    # contraction blocks b=(ci,dy) lex-ordered; pairs share dx (AP %16 rule)
    q3_t = [big.tile([128, NPIX2], BF, tag=f"q3{ct}", name=f"q3{ct}") for ct in range(CT)]
    blocks = [(ci, dy) for ci in range(CT) for dy in range(3)]
    for co in range(CT):
        for j in range(2):
            ps = ps_d.tile([128, 512], F32, tag="ps_d", name="ps_d")
            for dx in range(3):
                for p in range(4):
                    ci0, dy0 = blocks[2 * p]
                    ci1, dy1 = blocks[2 * p + 1]
                    m0 = ci0 * CB2 + (16 * j + dy0) * RW2 + dx
                    dm = (ci1 - ci0) * CB2 + (dy1 - dy0) * RW2
                    w0 = (ci0 * 9 + 3 * dy0 + dx) * DIM + co * 128
                    nc.tensor.matmul(
                        ps[:],
                        _ap(wqdwT_t[:], w0, [[3 * DIM, 2], [1, 128]]),
                        _ap(q1padf, m0, [[dm, 2], [RW2, 16], [1, 32]]),
                        start=(dx == 0 and p == 0), stop=False, perf_mode=DR)
                m8 = 2 * CB2 + (16 * j + 2) * RW2 + dx
                w8 = (2 * 9 + 6 + dx) * DIM + co * 128
                nc.tensor.matmul(
                    ps[:],
                    _ap(wqdwT_t[:], w8, [[1, 128]]),
                    _ap(q1padf, m8, [[RW2, 16], [1, 32]]),
                    start=False, stop=(dx == 2))
            nc.vector.tensor_copy(q3_t[co][:, 512 * j:512 * (j + 1)], ps[:])

    # ---------------- phase E: q norms + temperature ----------------
    for ct in range(CT):
        sq = wrk.tile([128, NPIX2], BF, tag="sqq", name="sqq")
        nrm2 = wrk.tile([128, 1], F32, tag="nrm2q", name="nrm2q")
        nc.scalar.activation(sq[:], q3_t[ct][:], AF.Square, bias=zero_col[:],
                             accum_out=nrm2[:])
        nrm = wrk.tile([128, 1], F32, tag="nrmq", name="nrmq")
        nc.scalar.activation(nrm[:], nrm2[:], AF.Sqrt, bias=eps_col[:])
        inv = wrk.tile([128, 1], F32, tag="invq", name="invq")
        nc.vector.reciprocal(inv[:], nrm[:])
        invt = wrk.tile([128, 1], F32, tag="invqt", name="invqt")
        nc.scalar.mul(invt[:], inv[:], tempc_t[ct][:])
        nc.vector.tensor_scalar_mul(q3_t[ct][:], q3_t[ct][:], invt[:])
        if "q3n" in dbg_d:
            nc.sync.dma_start(dbg_d["q3n"].ap()[128 * ct:128 * (ct + 1), :], q3_t[ct][:])

    # ---------------- phase E2: q3T via PE transpose ----------------
    q3T = [big.tile([128, DIM], BF, tag=f"q3T{pt}", name=f"q3T{pt}") for pt in range(8)]
    for ct in range(CT):
        for pt in range(8):
            pst = ps_m.tile([128, 128], BF, tag="ps_m", name="ps_m")
            nc.tensor.transpose(pst[:], q3_t[ct][:, 128 * pt:128 * (pt + 1)], idn_t[:])
            eng = (nc.vector.tensor_copy, nc.scalar.copy)[(ct + pt) % 2]
            eng(q3T[pt][:, 128 * ct:128 * (ct + 1)], pst[:])

    # ---------------- phase F: QK attn + softmax + M-build ----------------
    mst3 = big.tile([128, CT, DIM], BF, tag="mst3", name="mst3")
    att_n = []
    for h in range(HEADS):
        cs = slice(HC * h, HC * (h + 1))
        pa = ps_d.tile([HC, HC], F32, tag="ps_d", name="ps_d")
        for pt in range(8):
            nc.tensor.matmul(pa[:], q3T[pt][:, cs], kpT[pt][:, cs],
                             start=(pt == 0), stop=(pt == 7))
        ae = wrk.tile([HC, HC], BF, tag=f"ae{h % 2}", name=f"ae{h % 2}")
        nc.scalar.activation(ae[:], pa[:], AF.Exp, bias=zero_col[0:HC, :])
        zs = wrk.tile([HC, 1], F32, tag="zs", name="zs")
        nc.vector.tensor_reduce(zs[:], ae[:], axis=mybir.AxisListType.X, op=OP.add)
        zi = wrk.tile([HC, 1], F32, tag="zi", name="zi")
        nc.vector.reciprocal(zi[:], zs[:])
        an = wrk.tile([HC, HC], BF, tag=f"an{h}", name=f"an{h}")
        nc.vector.tensor_scalar_mul(an[:], ae[:], zi[:])
        att_n.append(an)
        if "att" in dbg_d:
            nc.sync.dma_start(dbg_d["att"].ap()[HC * h:HC * (h + 1), :], an[:])
    for h in range(HEADS):
        pm = ps_d.tile([HC, DIM], F32, tag="ps_d", name="ps_d")
        nc.tensor.matmul(pm[:], att_n[h][:], wpT_t[h][:], start=True, stop=True)
        stg = wrk.tile([HC, DIM], BF, tag=f"stg{h % 2}", name=f"stg{h % 2}")
        nc.vector.tensor_copy(stg[:], pm[:])
        g0 = HC * h
        t0, o0 = divmod(g0, 128)
        n0 = min(128 - o0, HC)
        nc.sync.dma_start(mst3[o0:o0 + n0, t0, :], stg[0:n0, :])
        if n0 < HC:
            nc.sync.dma_start(mst3[0:HC - n0, t0 + 1, :], stg[n0:HC, :])
    if "mst" in dbg_d:
        nc.sync.dma_start(dbg_d["mst"].ap(), mst3[:].rearrange("p a b -> p (a b)"))

    # ---------------- phases B1 + H interleaved by pixel group ------------
    # B1: v depthwise (bf16 diag matmuls); H: out = Mst.T @ v_dw (bf16),
    # g-group at a time so output DMA streams while later groups compute.
    v_dw3 = big.tile([128, CT, NPIX], BF, tag="v_dw3", name="v_dw3")
    v_dwf = v_dw3[:].rearrange("p a b -> p (a b)")
    mstf = mst3[:].rearrange("p a b -> p (a b)")
    # taps 3,4,5 (dy=1) run on scalar+vector, software-pipelined: chunk
    # c+1's tap products are emitted before chunk c's PSUM fold so the
    # vector queue never idles on the tensor's accumulation.
    pe_taps = [t9 for t9 in range(9) if t9 not in (3, 4, 5)]
    b1_tmp = {}

    def b1_build_tmp(ct, c):
        r0 = 8 * c
        tmp = wrk.tile([128, 8, 64], BF, tag=f"b1t{(ct * 8 + c) % 2}",
                       name="b1tmp")
        nc.scalar.mul(tmp[:], vpad[:, ct, 1 + r0:9 + r0, 0:64],
                      w3v_t[:, 9 * ct + 3:9 * ct + 4])
        nc.vector.scalar_tensor_tensor(
            out=tmp[:], in0=vpad[:, ct, 1 + r0:9 + r0, 1:65],
            scalar=w3v_t[:, 9 * ct + 4:9 * ct + 5], in1=tmp[:],
            op0=OP.mult, op1=OP.add)
        nc.vector.scalar_tensor_tensor(
            out=tmp[:], in0=vpad[:, ct, 1 + r0:9 + r0, 2:66],
            scalar=w3v_t[:, 9 * ct + 5:9 * ct + 6], in1=tmp[:],
            op0=OP.mult, op1=OP.add)
        b1_tmp[(ct, c)] = tmp

    for g in range(4):
        items = [(ct, 2 * g + jj) for ct in range(CT) for jj in range(2)]
        if (items[0]) not in b1_tmp:
            b1_build_tmp(*items[0])
        for i, (ct, c) in enumerate(items):
            r0 = 8 * c
            ps = ps_a.tile([128, 512], F32, tag="ps_a", name="ps_a")
            for k, t9 in enumerate(pe_taps):
                dy, dx = t9 // 3, t9 % 3
                nc.tensor.matmul(
                    ps[:],
                    dgv_t[:, ct, t9, :],
                    _ap(vpadf, ct * CB + (r0 + dy) * RW + dx, [[RW, 8], [1, 64]]),
                    start=(k == 0), stop=(k == len(pe_taps) - 1))
            if i + 1 < len(items):
                b1_build_tmp(*items[i + 1])
            elif g < 3:
                b1_build_tmp(items[0][0], 2 * (g + 1))
            tmp = b1_tmp.pop((ct, c))
            nc.vector.scalar_tensor_tensor(
                out=v_dw3[:, ct, 512 * c:512 * (c + 1)],
                in0=ps[:], scalar=1.0,
                in1=tmp[:].rearrange("p a b -> p (a b)"),
                op0=OP.mult, op1=OP.add)
        for ob in range(CT):
            ot = osb.tile([128, 1024], BF, tag="osb", name="osb")
            for jj in range(2):
                ch = 2 * g + jj
                ps = ps_a.tile([128, 512], F32, tag="ps_a", name="ps_a")
                for ctd in range(CT):
                    nc.tensor.matmul(
                        ps[:],
                        _ap(mstf, ctd * DIM + ob * 128, [[1, 128]]),
                        _ap(v_dwf, ctd * NPIX + ch * 512, [[1, 512]]),
                        start=(ctd == 0), stop=(ctd == CT - 1))
                nc.scalar.copy(ot[:, 512 * jj:512 * (jj + 1)], ps[:])
            deng = nc.sync if (g + ob) % 2 == 0 else nc.gpsimd
            deng.dma_start(out_d.ap()[128 * ob:128 * (ob + 1),
                                      1024 * g:1024 * (g + 1)], ot[:])
    if "vdw" in dbg_d:
        for ct in range(CT):
            nc.sync.dma_start(dbg_d["vdw"].ap()[128 * ct:128 * (ct + 1), :],
                              v_dw3[:, ct, :])
    ctx.close()


# ======================= host-side wrapper =======================

def _f8(a):
    return np.clip(a, -240.0, 240.0).astype(F8_NP)


def _prep_shared(w_kv, w_kv_dw, w_q, w_q_dw, w_proj, temperature):
    """Shared (replicated) weight preprocessing on host."""
    w_kv = np.asarray(w_kv, np.float32)[:, :, 0, 0]          # [768, 384]
    w_kv_dw = np.asarray(w_kv_dw, np.float32)[:, 0]          # [768, 3, 3]
    w_q = np.asarray(w_q, np.float32)[:, :, 0, 0]            # [384, 384]
    w_q_dw = np.asarray(w_q_dw, np.float32)                  # [384, 384, 3, 3]
    w_proj = np.asarray(w_proj, np.float32)[:, :, 0, 0]      # [384, 384]
    temperature = np.asarray(temperature, np.float32).reshape(HEADS)

    # wkT3[ki, ci, co] = w_kv[co, ci*128+ki] * S_A  (k half, fp8)
    wkT3 = np.transpose(
        (w_kv[:DIM] * S_A).reshape(DIM, CT, 128), (2, 1, 0)).reshape(128, -1)
    # wvT3: v half, bf16, natural scale
    wvT3 = np.transpose(
        w_kv[DIM:].reshape(DIM, CT, 128), (2, 1, 0)).reshape(128, -1)
    wqT3 = np.transpose(
        (w_q * S_A).reshape(DIM, CT, 128), (2, 1, 0)).reshape(128, -1)
    # wqdwT3[ki, ci, t, co] = w_q_dw[co, ci*128+ki, t//3, t%3] * S_DW
    wqdwT3 = np.transpose(
        (w_q_dw * S_DW).reshape(DIM, CT, 128, 9), (2, 1, 3, 0)).reshape(128, -1)

    w3v = w_kv_dw[DIM:].reshape(DIM, 9)                      # [384, 9] natural
    # fold 2x2 mean pool into k-half depthwise -> 4x4 stride-2 taps
    w3k = w_kv_dw[:DIM]
    w4k = np.zeros((DIM, 4, 4), np.float32)
    for uy in range(4):
        for ux in range(4):
            acc = np.zeros(DIM, np.float32)
            for dy in range(2):
                for dx in range(2):
                    ky, kx = uy - dy, ux - dx
                    if 0 <= ky < 3 and 0 <= kx < 3:
                        acc += w3k[:, ky, kx]
            w4k[:, uy, ux] = 0.25 * acc * S_DW
    # depthwise weight columns (for the engine-computed B1 taps) and diag
    # matrices (PE depthwise).  dgv layout [ki, ct, t9, 128]; dgk layout per
    # ct: ux blocks of 512 = DR pairs (uy0,1)+(uy2,3).
    w3vc = np.transpose(w3v.reshape(CT, 128, 9), (1, 0, 2)).reshape(128, -1)
    ii = np.arange(128)
    w3v_t = w3v.reshape(CT, 128, 9)
    w4k_t = w4k.reshape(CT, 128, 4, 4)
    dgv = np.zeros((128, CT, 9, 128), np.float32)
    dgk = np.zeros((128, CT, 2048), np.float32)
    for ct in range(CT):
        for t9 in range(9):
            dgv[ii, ct, t9, ii] = w3v_t[ct, :, t9]
        for ux in range(4):
            for pp in range(2):
                dgk[ii, ct, ux * 512 + pp * 256 + ii] = w4k_t[ct, :, 2 * pp, ux]
                dgk[ii, ct, ux * 512 + pp * 256 + 128 + ii] = w4k_t[ct, :, 2 * pp + 1, ux]

    wpT = np.ascontiguousarray(w_proj.T).astype(BF_NP)
    temp_col = np.repeat(temperature, HC)[:, None].astype(np.float32)
    idn = np.eye(128, dtype=BF_NP)
    return dict(wkT3=_f8(wkT3), wvT3=wvT3.astype(BF_NP), wqT3=_f8(wqT3),
                wqdwT3=_f8(wqdwT3), w3vc=w3vc.astype(np.float32),
                dgv=dgv.reshape(128, -1).astype(BF_NP),
                dgk=_f8(dgk.reshape(128, -1)),
                wpT=wpT, temp_col=temp_col, idn=idn)


_NC_CACHE = {}


def _get_nc(dbg=False):
    key = bool(dbg)
    if key not in _NC_CACHE:
        _NC_CACHE[key] = build_program(dbg=key)
    return _NC_CACHE[key]


def make_in_maps(x, y, shared):
    x = np.asarray(x, np.float32)
    y = np.asarray(y, np.float32)
    B = x.shape[0]
    in_maps = []
    for b in range(B):
        m = dict(shared)
        # xb3[ki, ci, p] = x[b, ci*128+ki, p]
        xt = np.transpose(x[b].reshape(CT, 128, NPIX), (1, 0, 2)).reshape(128, -1)
        m["xb3"] = xt.astype(BF_NP)
        m["x8"] = _f8(xt)
        m["y3"] = _f8(np.transpose(y[b].reshape(CT, 128, NPIX2), (1, 0, 2))
                      .reshape(128, -1))
        in_maps.append(m)
    return in_maps


def kernel(x, y, w_kv, w_kv_dw, w_q, w_q_dw, w_proj, temperature):
    nc = _get_nc(dbg=False)
    shared = _prep_shared(w_kv, w_kv_dw, w_q, w_q_dw, w_proj, temperature)
    in_maps = make_in_maps(x, y, shared)
    res = run_bass_kernel_spmd(nc, in_maps, core_ids=list(range(len(in_maps))))
    out = np.stack([r["out"].astype(np.float32).reshape(DIM, H, H)
                    for r in res.results])
    return out

